# revision 23
# baseline (speedup 1.0000x reference)
"""Batched 20x20 SPD covariance-matrix inversion on 8 Trainium2 NeuronCores.

For each of 131072 batches: build C = exp(-1.5 * pairwise_dist(pos)) + 0.01*I
from 20 2-D points, return C^{-1}.

Strategy (per core, data-parallel over batch):
 - batch-major layout: each of 128 SBUF partitions holds M matrices' full
   20x20 (400 fp32) in the free dim; CHUNKS chunks of M per partition.
 - symmetric sweep operator (Gauss-Jordan preserving symmetry): only the
   upper triangle is updated each pivot, covered by short row-band
   rectangles (pivot row excluded -- it is overwritten afterwards anyway);
   final negate + mirror emit the full inverse.
 - engine placement: ALL tensor-tensor work on DVE (GPSIMD shares its SBUF
   ports with DVE -- concurrent use slows DVE ~2x, measured), everything
   unary (square/sqrt/exp, gathers, pivot row/col writes, negate, mirror)
   on ACT.  DVE measures ~100% busy in this arrangement.
 - chunks are processed in pairs with the pivot loop interleaved between
   the two chunks so DVE always has independent work while the other
   chunk's serial pivot chain (ACT gather -> recip -> cr) resolves.
"""

import numpy as np

import concourse.bass as bass  # noqa: F401  (registers engine APIs)
import concourse.tile as tile
from concourse import bacc, mybir
from concourse.bass_utils import run_bass_kernel_spmd

N = 20                  # matrix dim
D = 2                   # coord dim
PHI = 1.5
TAU = 0.01
P = 128                 # SBUF partitions
N_CORES = 8
B_TOTAL = 131072
B_CORE = B_TOTAL // N_CORES   # 16384

F32 = mybir.dt.float32
AF = mybir.ActivationFunctionType
OP = mybir.AluOpType

RECT_H = 3              # row-band height of the triangle cover
FAST_RECIP = True


def pivot_rects(k):
    """Upper-triangle cover for pivot k: row bands [r0,r1) x cols [r0,N),
    excluding the pivot row k (its values are overwritten by the row copy).
    DVE per-instruction overhead is ~160ns; bands of RECT_H rows balance
    cover waste (rows above r0) against instruction count."""
    rects = []
    runs = []
    row = 0
    for row in range(N):
        if row == k:
            continue
        if runs and runs[-1][1] == row:
            runs[-1][1] = row + 1
        else:
            runs.append([row, row + 1])
    for (a, b) in runs:
        r = a
        while r < b:
            r1 = min(r + RECT_H, b)
            rects.append((r, r1))
            r = r1
    return rects


def emit_chunk_head(tc, pools, pos_r, c, m_chunk):
    """DMA in + covariance build for chunk c. Returns (A tile, A4 view)."""
    nc = tc.nc
    M = m_chunk
    H = N // 2
    pos_pool, a_pool, cov_pool = pools["pos"], pools["A"], pools["cov"]

    pos_t = pos_pool.tile([P, M * N * D], F32)
    nc.sync.dma_start(pos_t[:, :], pos_r[:, c, :])
    posv = pos_t[:, :].rearrange("p (m i d) -> p m i d", m=M, i=N)

    A = a_pool.tile([P, M * N * N], F32)
    A4 = A[:, :].rearrange("p (m i j) -> p m i j", m=M, i=N)

    # ---- covariance build: A = exp(-PHI * dist); diag is exactly 1.0 ----
    # ACT ops grouped by activation-table set (Square/Exp/Copy vs Sqrt):
    # interleaving Sqrt and Exp costs a 1283ns ACT_TABLE_LOAD per switch.
    regs = []
    for h in range(2):
        jsl = slice(h * H, (h + 1) * H)
        reg = A4[:, :, :, jsl]
        xi = posv[:, :, :, 0].unsqueeze(3).broadcast_to([P, M, N, H])
        xj = posv[:, :, jsl, 0].unsqueeze(2).broadcast_to([P, M, N, H])
        nc.vector.tensor_sub(reg, xi, xj)
        nc.scalar.square(reg, reg)
        dy = cov_pool.tile([P, M * N * H], F32, tag="dy")
        dyv = dy[:, :].rearrange("p (m i j) -> p m i j", m=M, i=N)
        yi = posv[:, :, :, 1].unsqueeze(3).broadcast_to([P, M, N, H])
        yj = posv[:, :, jsl, 1].unsqueeze(2).broadcast_to([P, M, N, H])
        nc.vector.tensor_sub(dyv, yi, yj)
        nc.scalar.square(dyv, dyv)
        nc.vector.tensor_add(reg, reg, dyv)
        regs.append(reg)
    for reg in regs:
        nc.scalar.sqrt(reg, reg)
    for reg in regs:
        nc.scalar.activation(reg, reg, AF.Exp, scale=-PHI)

    # diag <- (1+TAU): dist is exactly 0 there so exp gave exactly 1.0;
    # a Copy-with-scale on ACT turns it into 1+TAU without touching DVE.
    Av = A[:, :].rearrange("p (m x) -> p m x", m=M)
    diag = Av[:, :, 0 : N * N : N + 1]
    nc.scalar.mul(diag, diag, 1.0 + TAU)
    return A, A4


def emit_gather(tc, pools, A4, k, m_chunk, slot):
    """ACT: assemble pivot column k into fresh c/cr/r tiles for `slot`.
    For k=0 the matrix is still symmetric and row 0 is never written by the
    rects (pivot row excluded), so c is just a view of row 0 -- no copy."""
    nc = tc.nc
    M = m_chunk
    small_pool = pools["small"]
    crK = small_pool.tile([P, M * N], F32, tag=f"cr{slot}")
    rK = small_pool.tile([P, M], F32, tag=f"r{slot}")
    cr3 = crK[:, :].rearrange("p (m i) -> p m i", m=M)
    if k == 0:
        return {"c3": A4[:, :, 0, :], "cr3": cr3, "rK": rK}
    cK = small_pool.tile([P, M * N], F32, tag=f"c{slot}")
    c3 = cK[:, :].rearrange("p (m i) -> p m i", m=M)
    nc.scalar.copy(c3[:, :, :k], A4[:, :, :k, k])
    nc.scalar.copy(c3[:, :, k:], A4[:, :, k, k:])
    return {"c3": c3, "cr3": cr3, "rK": rK}


def emit_work(tc, pools, A4, k, m_chunk, slot, st):
    """DVE rank-1 update + ACT row/col/diag writes for pivot k."""
    nc = tc.nc
    M = m_chunk
    rect_pool = pools["rect"]
    c3, cr3, rK = st["c3"], st["cr3"], st["rK"]

    if FAST_RECIP:
        nc.vector.reciprocal_approx_fast(rK[:, :], c3[:, :, k])
    else:
        nc.vector.reciprocal(rK[:, :], c3[:, :, k])
    rb = rK[:, :].unsqueeze(2).broadcast_to([P, M, N])
    nc.vector.tensor_mul(cr3, c3, rb)

    # rank-1 update of the upper triangle (pivot row k excluded; column k
    # gets garbage that the column copy below overwrites)
    max_area = RECT_H * N
    for (r0, r1) in pivot_rects(k):
        nr, ncl = r1 - r0, N - r0
        tmp = rect_pool.tile([P, M * max_area], F32, tag=f"rect{slot}")
        tv = tmp[:, : M * nr * ncl].rearrange("p (m i j) -> p m i j", m=M, i=nr)
        cb = c3[:, :, r0:r1].unsqueeze(3).broadcast_to([P, M, nr, ncl])
        crb = cr3[:, :, r0:].unsqueeze(2).broadcast_to([P, M, nr, ncl])
        nc.vector.tensor_mul(tv, cb, crb)
        reg = A4[:, :, r0:r1, r0:]
        nc.vector.tensor_sub(reg, reg, tv)

    # pivot row/col (upper parts) <- cr, diag <- -r (after the rects)
    if k:
        nc.scalar.copy(A4[:, :, :k, k], cr3[:, :, :k])
    if k < N - 1:
        nc.scalar.copy(A4[:, :, k, k + 1 :], cr3[:, :, k + 1 :])
    nc.scalar.mul(A4[:, :, k, k], rK[:, :], -1.0)


def emit_chunk_tail(tc, pools, A, A4, out_r, c, m_chunk, on_dve=False):
    """Emit the negated full inverse into separate output tiles (upper via a
    scale=-1 copy, lower via mirrored scale=-1 copies), then DMA out.  Going
    through out-tiles (not in-place) frees the A buffer as soon as the reads
    finish -- the next pair's cov build doesn't wait for the DMA.  on_dve
    runs the copies on DVE -- used for the very last chunk, where DVE would
    otherwise sit idle."""
    nc = tc.nc
    M = m_chunk
    Mh = M // 2
    half = M * N * N // 2
    for h in range(2):
        pool = pools["out"] if h == 0 else pools["cov"]
        oh_t = pool.tile([P, half], F32, tag="out" if h == 0 else "dy")
        oh = oh_t[:, :]
        oh4 = oh.rearrange("p (m i j) -> p m i j", m=Mh, i=N)
        Ah = A[:, h * half : (h + 1) * half]
        Ah4 = Ah.rearrange("p (m i j) -> p m i j", m=Mh, i=N)
        if on_dve:
            nc.vector.tensor_scalar_mul(oh, Ah, -1.0)
            for i in range(N - 1):
                nc.vector.tensor_scalar_mul(
                    oh4[:, :, i + 1 :, i], Ah4[:, :, i, i + 1 :], -1.0
                )
        else:
            nc.scalar.mul(oh, Ah, -1.0)
            for i in range(N - 1):
                nc.scalar.mul(oh4[:, :, i + 1 :, i], Ah4[:, :, i, i + 1 :], -1.0)
        nc.sync.dma_start(out_r[:, c, h * half : (h + 1) * half], oh)


def emit_kernel(tc, pos_ap, out_ap, b_core, m_chunk):
    """Emit the per-core program. pos: [b_core, 40] f32, out: [b_core, 400] f32."""
    chunks = b_core // (P * m_chunk)
    assert b_core == P * m_chunk * chunks
    assert chunks % 2 == 0

    pos_r = pos_ap.rearrange("(p c m) f -> p c (m f)", p=P, c=chunks)
    out_r = out_ap.rearrange("(p c m) f -> p c (m f)", p=P, c=chunks)

    with (
        tc.tile_pool(name="pos", bufs=2) as pos_pool,
        tc.tile_pool(name="A", bufs=2) as a_pool,
        tc.tile_pool(name="cov", bufs=1) as cov_pool,
        tc.tile_pool(name="rect", bufs=1) as rect_pool,
        tc.tile_pool(name="small", bufs=1) as small_pool,
        tc.tile_pool(name="out", bufs=1) as out_pool,
    ):
        pools = {
            "pos": pos_pool,
            "A": a_pool,
            "cov": cov_pool,
            "rect": rect_pool,
            "small": small_pool,
            "out": out_pool,
        }
        last = chunks // 2 - 1
        for pair in range(chunks // 2):
            ca, cb = 2 * pair, 2 * pair + 1
            Aa, A4a = emit_chunk_head(tc, pools, pos_r, ca, m_chunk)
            Ab, A4b = emit_chunk_head(tc, pools, pos_r, cb, m_chunk)
            for k in range(N):
                sta = emit_gather(tc, pools, A4a, k, m_chunk, 0)
                emit_work(tc, pools, A4a, k, m_chunk, 0, sta)
                stb = emit_gather(tc, pools, A4b, k, m_chunk, 1)
                emit_work(tc, pools, A4b, k, m_chunk, 1, stb)
            emit_chunk_tail(tc, pools, Aa, A4a, out_r, ca, m_chunk)
            emit_chunk_tail(
                tc, pools, Ab, A4b, out_r, cb, m_chunk, on_dve=(pair == last)
            )


_CACHE = {}


def build_nc(b_core=B_CORE, m_chunk=32, num_devices=N_CORES):
    key = (b_core, m_chunk, num_devices)
    if key in _CACHE:
        return _CACHE[key]
    nc = bacc.Bacc(
        "TRN2", target_bir_lowering=False, debug=False, num_devices=num_devices
    )
    pos_d = nc.dram_tensor("pos", [b_core, N * D], F32, kind="ExternalInput")
    out_d = nc.dram_tensor("out", [b_core, N * N], F32, kind="ExternalOutput")
    with tile.TileContext(nc) as tc:
        emit_kernel(tc, pos_d.ap(), out_d.ap(), b_core, m_chunk)
    nc.compile()
    _CACHE[key] = nc
    return nc


def run(pos_full, b_core=B_CORE, m_chunk=32, n_cores=N_CORES, **kw):
    """pos_full: [n_cores*b_core, 20, 2] f32 -> [n_cores*b_core, 20, 20] f32."""
    nc = build_nc(b_core, m_chunk, n_cores)
    flat = np.ascontiguousarray(
        np.asarray(pos_full, dtype=np.float32).reshape(-1, N * D)
    )
    in_maps = [
        {"pos": flat[i * b_core : (i + 1) * b_core]} for i in range(n_cores)
    ]
    res = run_bass_kernel_spmd(nc, in_maps, core_ids=list(range(n_cores)), **kw)
    out = np.concatenate([r["out"] for r in res.results], axis=0)
    return out.reshape(-1, N, N), res


def kernel(neighbor_positions, edge_list=None):
    out, _ = run(neighbor_positions)
    return out


# revision 24
# speedup vs baseline: 1.0053x; 1.0053x over previous
"""Batched 20x20 SPD covariance-matrix inversion on 8 Trainium2 NeuronCores.

For each of 131072 batches: build C = exp(-1.5 * pairwise_dist(pos)) + 0.01*I
from 20 2-D points, return C^{-1}.

Strategy (per core, data-parallel over batch):
 - batch-major layout: each of 128 SBUF partitions holds M matrices' full
   20x20 (400 fp32) in the free dim; CHUNKS chunks of M per partition.
 - symmetric sweep operator (Gauss-Jordan preserving symmetry): only the
   upper triangle is updated each pivot, covered by short row-band
   rectangles (pivot row excluded -- it is overwritten afterwards anyway);
   final negate + mirror emit the full inverse.
 - engine placement: ALL tensor-tensor work on DVE (GPSIMD shares its SBUF
   ports with DVE -- concurrent use slows DVE ~2x, measured), everything
   unary (square/sqrt/exp, gathers, pivot row/col writes, negate, mirror)
   on ACT.  DVE measures ~100% busy in this arrangement.
 - chunks are processed in pairs with the pivot loop interleaved between
   the two chunks so DVE always has independent work while the other
   chunk's serial pivot chain (ACT gather -> recip -> cr) resolves.
"""

import numpy as np

import concourse.bass as bass  # noqa: F401  (registers engine APIs)
import concourse.tile as tile
from concourse import bacc, mybir
from concourse.bass_utils import run_bass_kernel_spmd

N = 20                  # matrix dim
D = 2                   # coord dim
PHI = 1.5
TAU = 0.01
P = 128                 # SBUF partitions
N_CORES = 8
B_TOTAL = 131072
B_CORE = B_TOTAL // N_CORES   # 16384

F32 = mybir.dt.float32
AF = mybir.ActivationFunctionType
OP = mybir.AluOpType

RECT_H = 3              # row-band height of the triangle cover
FAST_RECIP = True


def pivot_rects(k):
    """Upper-triangle cover for pivot k: row bands [r0,r1) x cols [r0,N),
    excluding the pivot row k (its values are overwritten by the row copy).
    DVE per-instruction overhead is ~160ns; bands of RECT_H rows balance
    cover waste (rows above r0) against instruction count."""
    rects = []
    runs = []
    row = 0
    for row in range(N):
        if row == k:
            continue
        if runs and runs[-1][1] == row:
            runs[-1][1] = row + 1
        else:
            runs.append([row, row + 1])
    for (a, b) in runs:
        r = a
        while r < b:
            r1 = min(r + RECT_H, b)
            rects.append((r, r1))
            r = r1
    return rects


def emit_chunk_head(tc, pools, pos_r, c, m_chunk):
    """DMA in + covariance build for chunk c. Returns (A tile, A4 view)."""
    nc = tc.nc
    M = m_chunk
    H = N // 2
    pos_pool, a_pool, cov_pool = pools["pos"], pools["A"], pools["cov"]

    pos_t = pos_pool.tile([P, M * N * D], F32)
    nc.sync.dma_start(pos_t[:, :], pos_r[:, c, :])
    posv = pos_t[:, :].rearrange("p (m i d) -> p m i d", m=M, i=N)

    A = a_pool.tile([P, M * N * N], F32)
    A4 = A[:, :].rearrange("p (m i j) -> p m i j", m=M, i=N)

    # ---- covariance build: A = exp(-PHI * dist); diag is exactly 1.0 ----
    # ACT ops grouped by activation-table set (Square/Exp/Copy vs Sqrt):
    # interleaving Sqrt and Exp costs a 1283ns ACT_TABLE_LOAD per switch.
    regs = []
    for h in range(2):
        jsl = slice(h * H, (h + 1) * H)
        reg = A4[:, :, :, jsl]
        xi = posv[:, :, :, 0].unsqueeze(3).broadcast_to([P, M, N, H])
        xj = posv[:, :, jsl, 0].unsqueeze(2).broadcast_to([P, M, N, H])
        nc.vector.tensor_sub(reg, xi, xj)
        nc.scalar.square(reg, reg)
        dy = cov_pool.tile([P, M * N * H], F32, tag="dy")
        dyv = dy[:, :].rearrange("p (m i j) -> p m i j", m=M, i=N)
        yi = posv[:, :, :, 1].unsqueeze(3).broadcast_to([P, M, N, H])
        yj = posv[:, :, jsl, 1].unsqueeze(2).broadcast_to([P, M, N, H])
        nc.vector.tensor_sub(dyv, yi, yj)
        nc.scalar.square(dyv, dyv)
        nc.vector.tensor_add(reg, reg, dyv)
        regs.append(reg)
    for reg in regs:
        nc.scalar.sqrt(reg, reg)
    for reg in regs:
        nc.scalar.activation(reg, reg, AF.Exp, scale=-PHI)

    # diag <- (1+TAU): dist is exactly 0 there so exp gave exactly 1.0;
    # a Copy-with-scale on ACT turns it into 1+TAU without touching DVE.
    Av = A[:, :].rearrange("p (m x) -> p m x", m=M)
    diag = Av[:, :, 0 : N * N : N + 1]
    nc.scalar.mul(diag, diag, 1.0 + TAU)
    return A, A4


def emit_gather(tc, pools, A4, k, m_chunk, slot):
    """ACT: assemble pivot column k into fresh c/cr/r tiles for `slot`.
    For k=0 the matrix is still symmetric and row 0 is never written by the
    rects (pivot row excluded), so c is just a view of row 0 -- no copy."""
    nc = tc.nc
    M = m_chunk
    small_pool = pools["small"]
    crK = small_pool.tile([P, M * N], F32, tag=f"cr{slot}")
    rK = small_pool.tile([P, M], F32, tag=f"r{slot}")
    cr3 = crK[:, :].rearrange("p (m i) -> p m i", m=M)
    if k == 0:
        return {"c3": A4[:, :, 0, :], "cr3": cr3, "rK": rK}
    cK = small_pool.tile([P, M * N], F32, tag=f"c{slot}")
    c3 = cK[:, :].rearrange("p (m i) -> p m i", m=M)
    nc.scalar.copy(c3[:, :, :k], A4[:, :, :k, k])
    nc.scalar.copy(c3[:, :, k:], A4[:, :, k, k:])
    return {"c3": c3, "cr3": cr3, "rK": rK}


def emit_work(tc, pools, A4, k, m_chunk, slot, st):
    """DVE rank-1 update + ACT row/col/diag writes for pivot k."""
    nc = tc.nc
    M = m_chunk
    rect_pool = pools["rect"]
    c3, cr3, rK = st["c3"], st["cr3"], st["rK"]

    if FAST_RECIP:
        nc.vector.reciprocal_approx_fast(rK[:, :], c3[:, :, k])
    else:
        nc.vector.reciprocal(rK[:, :], c3[:, :, k])
    rb = rK[:, :].unsqueeze(2).broadcast_to([P, M, N])
    nc.vector.tensor_mul(cr3, c3, rb)

    # rank-1 update of the upper triangle (pivot row k excluded; column k
    # gets garbage that the column copy below overwrites)
    max_area = RECT_H * N
    for (r0, r1) in pivot_rects(k):
        nr, ncl = r1 - r0, N - r0
        tmp = rect_pool.tile([P, M * max_area], F32, tag=f"rect{slot}")
        tv = tmp[:, : M * nr * ncl].rearrange("p (m i j) -> p m i j", m=M, i=nr)
        cb = c3[:, :, r0:r1].unsqueeze(3).broadcast_to([P, M, nr, ncl])
        crb = cr3[:, :, r0:].unsqueeze(2).broadcast_to([P, M, nr, ncl])
        nc.vector.tensor_mul(tv, cb, crb)
        reg = A4[:, :, r0:r1, r0:]
        nc.vector.tensor_sub(reg, reg, tv)

    # pivot row/col (upper parts) <- cr, diag <- -r (after the rects)
    if k:
        nc.scalar.copy(A4[:, :, :k, k], cr3[:, :, :k])
    if k < N - 1:
        nc.scalar.copy(A4[:, :, k, k + 1 :], cr3[:, :, k + 1 :])
    nc.scalar.mul(A4[:, :, k, k], rK[:, :], -1.0)


def emit_chunk_tail(tc, pools, A, A4, out_r, c, m_chunk, on_dve=False):
    """negate + mirror lower + DMA out for chunk c.  The two half-DMAs are
    issued from different engines (sync + ACT) so they land on separate
    HW-DGE queues and run in parallel.  on_dve (used only for the very last
    chunk, where DVE would otherwise idle) stages the negated result through
    out-tiles on DVE so the final DMAs start as early as possible."""
    nc = tc.nc
    M = m_chunk
    half = M * N * N // 2
    if on_dve:
        Mh = M // 2
        for h in range(2):
            pool = pools["out"] if h == 0 else pools["cov"]
            oh_t = pool.tile([P, half], F32, tag="out" if h == 0 else "dy")
            oh = oh_t[:, :]
            oh4 = oh.rearrange("p (m i j) -> p m i j", m=Mh, i=N)
            Ah = A[:, h * half : (h + 1) * half]
            Ah4 = Ah.rearrange("p (m i j) -> p m i j", m=Mh, i=N)
            nc.vector.tensor_scalar_mul(oh, Ah, -1.0)
            for i in range(N - 1):
                nc.vector.tensor_scalar_mul(
                    oh4[:, :, i + 1 :, i], Ah4[:, :, i, i + 1 :], -1.0
                )
            eng = nc.sync if h == 0 else nc.scalar
            eng.dma_start(out_r[:, c, h * half : (h + 1) * half], oh)
        return
    nc.scalar.mul(A[:, :], A[:, :], -1.0)
    for i in range(N - 1):
        nc.scalar.copy(A4[:, :, i + 1 :, i], A4[:, :, i, i + 1 :])
    nc.sync.dma_start(out_r[:, c, :half], A[:, :half])
    nc.scalar.dma_start(out_r[:, c, half:], A[:, half:])


def emit_kernel(tc, pos_ap, out_ap, b_core, m_chunk):
    """Emit the per-core program. pos: [b_core, 40] f32, out: [b_core, 400] f32."""
    chunks = b_core // (P * m_chunk)
    assert b_core == P * m_chunk * chunks
    assert chunks % 2 == 0

    pos_r = pos_ap.rearrange("(p c m) f -> p c (m f)", p=P, c=chunks)
    out_r = out_ap.rearrange("(p c m) f -> p c (m f)", p=P, c=chunks)

    with (
        tc.tile_pool(name="pos", bufs=2) as pos_pool,
        tc.tile_pool(name="A", bufs=2) as a_pool,
        tc.tile_pool(name="cov", bufs=1) as cov_pool,
        tc.tile_pool(name="rect", bufs=1) as rect_pool,
        tc.tile_pool(name="small", bufs=1) as small_pool,
        tc.tile_pool(name="out", bufs=1) as out_pool,
    ):
        pools = {
            "pos": pos_pool,
            "A": a_pool,
            "cov": cov_pool,
            "rect": rect_pool,
            "small": small_pool,
            "out": out_pool,
        }
        last = chunks // 2 - 1
        for pair in range(chunks // 2):
            ca, cb = 2 * pair, 2 * pair + 1
            Aa, A4a = emit_chunk_head(tc, pools, pos_r, ca, m_chunk)
            Ab, A4b = emit_chunk_head(tc, pools, pos_r, cb, m_chunk)
            for k in range(N):
                sta = emit_gather(tc, pools, A4a, k, m_chunk, 0)
                emit_work(tc, pools, A4a, k, m_chunk, 0, sta)
                stb = emit_gather(tc, pools, A4b, k, m_chunk, 1)
                emit_work(tc, pools, A4b, k, m_chunk, 1, stb)
            emit_chunk_tail(tc, pools, Aa, A4a, out_r, ca, m_chunk)
            emit_chunk_tail(
                tc, pools, Ab, A4b, out_r, cb, m_chunk, on_dve=(pair == last)
            )


_CACHE = {}


def build_nc(b_core=B_CORE, m_chunk=32, num_devices=N_CORES):
    key = (b_core, m_chunk, num_devices)
    if key in _CACHE:
        return _CACHE[key]
    nc = bacc.Bacc(
        "TRN2", target_bir_lowering=False, debug=False, num_devices=num_devices
    )
    pos_d = nc.dram_tensor("pos", [b_core, N * D], F32, kind="ExternalInput")
    out_d = nc.dram_tensor("out", [b_core, N * N], F32, kind="ExternalOutput")
    with tile.TileContext(nc) as tc:
        emit_kernel(tc, pos_d.ap(), out_d.ap(), b_core, m_chunk)
    nc.compile()
    _CACHE[key] = nc
    return nc


def run(pos_full, b_core=B_CORE, m_chunk=32, n_cores=N_CORES, **kw):
    """pos_full: [n_cores*b_core, 20, 2] f32 -> [n_cores*b_core, 20, 20] f32."""
    nc = build_nc(b_core, m_chunk, n_cores)
    flat = np.ascontiguousarray(
        np.asarray(pos_full, dtype=np.float32).reshape(-1, N * D)
    )
    in_maps = [
        {"pos": flat[i * b_core : (i + 1) * b_core]} for i in range(n_cores)
    ]
    res = run_bass_kernel_spmd(nc, in_maps, core_ids=list(range(n_cores)), **kw)
    out = np.concatenate([r["out"] for r in res.results], axis=0)
    return out.reshape(-1, N, N), res


def kernel(neighbor_positions, edge_list=None):
    out, _ = run(neighbor_positions)
    return out


# revision 27
# speedup vs baseline: 1.0269x; 1.0216x over previous
"""Batched 20x20 SPD covariance-matrix inversion on 8 Trainium2 NeuronCores.

For each of 131072 batches: build C = exp(-1.5 * pairwise_dist(pos)) + 0.01*I
from 20 2-D points, return C^{-1}.

Strategy (per core, data-parallel over batch):
 - batch-major layout: each of 128 SBUF partitions holds M matrices' full
   20x20 (400 fp32) in the free dim; CHUNKS chunks of M per partition.
 - symmetric sweep operator (Gauss-Jordan preserving symmetry): only the
   upper triangle is updated each pivot, covered by short row-band
   rectangles (pivot row excluded -- it is overwritten afterwards anyway);
   final negate + mirror emit the full inverse.
 - engine placement: ALL tensor-tensor work on DVE (GPSIMD shares its SBUF
   ports with DVE -- concurrent use slows DVE ~2x, measured), everything
   unary (square/sqrt/exp, gathers, pivot row/col writes, negate, mirror)
   on ACT.  DVE measures ~100% busy in this arrangement.
 - chunks are processed in pairs with the pivot loop interleaved between
   the two chunks so DVE always has independent work while the other
   chunk's serial pivot chain (ACT gather -> recip -> cr) resolves.
"""

import numpy as np

import concourse.bass as bass  # noqa: F401  (registers engine APIs)
import concourse.tile as tile
from concourse import bacc, mybir
from concourse.bass_utils import run_bass_kernel_spmd

N = 20                  # matrix dim
D = 2                   # coord dim
PHI = 1.5
TAU = 0.01
P = 128                 # SBUF partitions
N_CORES = 8
B_TOTAL = 131072
B_CORE = B_TOTAL // N_CORES   # 16384

F32 = mybir.dt.float32
AF = mybir.ActivationFunctionType
OP = mybir.AluOpType

RECT_H = 3              # row-band height of the triangle cover
FAST_RECIP = True


def pivot_rects(k):
    """Upper-triangle cover for pivot k: row bands [r0,r1) x cols [r0,N),
    excluding the pivot row k (its values are overwritten by the row copy).
    DVE per-instruction overhead is ~160ns; bands of RECT_H rows balance
    cover waste (rows above r0) against instruction count."""
    rects = []
    runs = []
    row = 0
    for row in range(N):
        if row == k:
            continue
        if runs and runs[-1][1] == row:
            runs[-1][1] = row + 1
        else:
            runs.append([row, row + 1])
    for (a, b) in runs:
        r = a
        while r < b:
            r1 = min(r + RECT_H, b)
            rects.append((r, r1))
            r = r1
    return rects


def emit_chunk_head(tc, pools, pos_r, c, m_chunk):
    """DMA in + covariance build for chunk c. Returns (A tile, A4 view)."""
    nc = tc.nc
    M = m_chunk
    H = N // 2
    pos_pool, a_pool, cov_pool = pools["pos"], pools["A"], pools["cov"]

    pos_t = pos_pool.tile([P, M * N * D], F32)
    nc.sync.dma_start(pos_t[:, :], pos_r[:, c, :])
    posv = pos_t[:, :].rearrange("p (m i d) -> p m i d", m=M, i=N)

    A = a_pool.tile([P, M * N * N], F32)
    A4 = A[:, :].rearrange("p (m i j) -> p m i j", m=M, i=N)

    # ---- covariance build: A = exp(-PHI * dist); diag is exactly 1.0 ----
    # ACT ops grouped by activation-table set (Square/Exp/Copy vs Sqrt):
    # interleaving Sqrt and Exp costs a 1283ns ACT_TABLE_LOAD per switch.
    regs = []
    for h in range(2):
        jsl = slice(h * H, (h + 1) * H)
        reg = A4[:, :, :, jsl]
        xi = posv[:, :, :, 0].unsqueeze(3).broadcast_to([P, M, N, H])
        xj = posv[:, :, jsl, 0].unsqueeze(2).broadcast_to([P, M, N, H])
        nc.vector.tensor_sub(reg, xi, xj)
        nc.scalar.square(reg, reg)
        dy = cov_pool.tile([P, M * N * H], F32, tag="dy")
        dyv = dy[:, :].rearrange("p (m i j) -> p m i j", m=M, i=N)
        yi = posv[:, :, :, 1].unsqueeze(3).broadcast_to([P, M, N, H])
        yj = posv[:, :, jsl, 1].unsqueeze(2).broadcast_to([P, M, N, H])
        nc.vector.tensor_sub(dyv, yi, yj)
        nc.scalar.square(dyv, dyv)
        nc.vector.tensor_add(reg, reg, dyv)
        regs.append(reg)
    for reg in regs:
        nc.scalar.sqrt(reg, reg)
    for reg in regs:
        nc.scalar.activation(reg, reg, AF.Exp, scale=-PHI)

    # diag <- (1+TAU): dist is exactly 0 there so exp gave exactly 1.0;
    # a Copy-with-scale on ACT turns it into 1+TAU without touching DVE.
    Av = A[:, :].rearrange("p (m x) -> p m x", m=M)
    diag = Av[:, :, 0 : N * N : N + 1]
    nc.scalar.mul(diag, diag, 1.0 + TAU)
    return A, A4


def emit_gather(tc, pools, A4, k, m_chunk, slot):
    """ACT: assemble pivot column k into fresh c/cr/r tiles for `slot`.
    For k=0 the matrix is still symmetric and row 0 is never written by the
    rects (pivot row excluded), so c is just a view of row 0 -- no copy."""
    nc = tc.nc
    M = m_chunk
    small_pool = pools["small"]
    crK = small_pool.tile([P, M * N], F32, tag=f"cr{slot}")
    rK = small_pool.tile([P, M], F32, tag=f"r{slot}")
    cr3 = crK[:, :].rearrange("p (m i) -> p m i", m=M)
    if k == 0:
        return {"c3": A4[:, :, 0, :], "cr3": cr3, "rK": rK}
    cK = small_pool.tile([P, M * N], F32, tag=f"c{slot}")
    c3 = cK[:, :].rearrange("p (m i) -> p m i", m=M)
    nc.scalar.copy(c3[:, :, :k], A4[:, :, :k, k])
    nc.scalar.copy(c3[:, :, k:], A4[:, :, k, k:])
    return {"c3": c3, "cr3": cr3, "rK": rK}


def emit_work(tc, pools, A4, k, m_chunk, slot, st):
    """DVE rank-1 update + ACT row/col/diag writes for pivot k.

    The sweep of all 20 pivots yields -A^{-1}; instead of a final negate
    pass, the LAST pivot writes negated values directly: its subtract is
    reversed (A <- t - A) and its row/col/diag writes are scaled by -1."""
    nc = tc.nc
    M = m_chunk
    last = k == N - 1
    rect_pool = pools["rect"]
    c3, cr3, rK = st["c3"], st["cr3"], st["rK"]

    if FAST_RECIP:
        nc.vector.reciprocal_approx_fast(rK[:, :], c3[:, :, k])
    else:
        nc.vector.reciprocal(rK[:, :], c3[:, :, k])
    rb = rK[:, :].unsqueeze(2).broadcast_to([P, M, N])
    nc.vector.tensor_mul(cr3, c3, rb)

    # rank-1 update of the upper triangle (pivot row k excluded; column k
    # gets garbage that the column copy below overwrites)
    max_area = RECT_H * N
    for (r0, r1) in pivot_rects(k):
        nr, ncl = r1 - r0, N - r0
        tmp = rect_pool.tile([P, M * max_area], F32, tag=f"rect{slot}")
        tv = tmp[:, : M * nr * ncl].rearrange("p (m i j) -> p m i j", m=M, i=nr)
        cb = c3[:, :, r0:r1].unsqueeze(3).broadcast_to([P, M, nr, ncl])
        crb = cr3[:, :, r0:].unsqueeze(2).broadcast_to([P, M, nr, ncl])
        nc.vector.tensor_mul(tv, cb, crb)
        reg = A4[:, :, r0:r1, r0:]
        if last:
            nc.vector.tensor_sub(reg, tv, reg)
        else:
            nc.vector.tensor_sub(reg, reg, tv)

    # pivot row/col (upper parts) <- +-cr, diag <- -+r (after the rects)
    s = -1.0 if last else 1.0
    if k:
        nc.scalar.mul(A4[:, :, :k, k], cr3[:, :, :k], s)
    if k < N - 1:
        nc.scalar.mul(A4[:, :, k, k + 1 :], cr3[:, :, k + 1 :], s)
    nc.scalar.mul(A4[:, :, k, k], rK[:, :], -s)


def emit_chunk_tail(tc, pools, A, A4, out_r, c, m_chunk, on_dve=False):
    """mirror lower <- upper (the sweep already wrote the negated inverse
    into the upper triangle) + DMA out, split in two.  on_dve mirrors on
    DVE -- used for the very last chunk, where DVE would otherwise idle."""
    nc = tc.nc
    half = m_chunk * N * N // 2
    if on_dve:
        for i in range(N - 1):
            nc.vector.tensor_copy(A4[:, :, i + 1 :, i], A4[:, :, i, i + 1 :])
    else:
        for i in range(N - 1):
            nc.scalar.copy(A4[:, :, i + 1 :, i], A4[:, :, i, i + 1 :])
    nc.sync.dma_start(out_r[:, c, :half], A[:, :half])
    nc.sync.dma_start(out_r[:, c, half:], A[:, half:])


def emit_kernel(tc, pos_ap, out_ap, b_core, m_chunk):
    """Emit the per-core program. pos: [b_core, 40] f32, out: [b_core, 400] f32."""
    chunks = b_core // (P * m_chunk)
    assert b_core == P * m_chunk * chunks
    assert chunks % 2 == 0

    pos_r = pos_ap.rearrange("(p c m) f -> p c (m f)", p=P, c=chunks)
    out_r = out_ap.rearrange("(p c m) f -> p c (m f)", p=P, c=chunks)

    with (
        tc.tile_pool(name="pos", bufs=2) as pos_pool,
        tc.tile_pool(name="A", bufs=2) as a_pool,
        tc.tile_pool(name="cov", bufs=1) as cov_pool,
        tc.tile_pool(name="rect", bufs=1) as rect_pool,
        tc.tile_pool(name="small", bufs=1) as small_pool,
    ):
        pools = {
            "pos": pos_pool,
            "A": a_pool,
            "cov": cov_pool,
            "rect": rect_pool,
            "small": small_pool,
        }
        last = chunks // 2 - 1
        for pair in range(chunks // 2):
            ca, cb = 2 * pair, 2 * pair + 1
            Aa, A4a = emit_chunk_head(tc, pools, pos_r, ca, m_chunk)
            Ab, A4b = emit_chunk_head(tc, pools, pos_r, cb, m_chunk)
            for k in range(N):
                sta = emit_gather(tc, pools, A4a, k, m_chunk, 0)
                emit_work(tc, pools, A4a, k, m_chunk, 0, sta)
                stb = emit_gather(tc, pools, A4b, k, m_chunk, 1)
                emit_work(tc, pools, A4b, k, m_chunk, 1, stb)
            emit_chunk_tail(tc, pools, Aa, A4a, out_r, ca, m_chunk)
            emit_chunk_tail(
                tc, pools, Ab, A4b, out_r, cb, m_chunk, on_dve=(pair == last)
            )


_CACHE = {}


def build_nc(b_core=B_CORE, m_chunk=32, num_devices=N_CORES):
    key = (b_core, m_chunk, num_devices)
    if key in _CACHE:
        return _CACHE[key]
    nc = bacc.Bacc(
        "TRN2", target_bir_lowering=False, debug=False, num_devices=num_devices
    )
    pos_d = nc.dram_tensor("pos", [b_core, N * D], F32, kind="ExternalInput")
    out_d = nc.dram_tensor("out", [b_core, N * N], F32, kind="ExternalOutput")
    with tile.TileContext(nc) as tc:
        emit_kernel(tc, pos_d.ap(), out_d.ap(), b_core, m_chunk)
    nc.compile()
    _CACHE[key] = nc
    return nc


def run(pos_full, b_core=B_CORE, m_chunk=32, n_cores=N_CORES, **kw):
    """pos_full: [n_cores*b_core, 20, 2] f32 -> [n_cores*b_core, 20, 20] f32."""
    nc = build_nc(b_core, m_chunk, n_cores)
    flat = np.ascontiguousarray(
        np.asarray(pos_full, dtype=np.float32).reshape(-1, N * D)
    )
    in_maps = [
        {"pos": flat[i * b_core : (i + 1) * b_core]} for i in range(n_cores)
    ]
    res = run_bass_kernel_spmd(nc, in_maps, core_ids=list(range(n_cores)), **kw)
    out = np.concatenate([r["out"] for r in res.results], axis=0)
    return out.reshape(-1, N, N), res


def kernel(neighbor_positions, edge_list=None):
    out, _ = run(neighbor_positions)
    return out
